# revision 36
# baseline (speedup 1.0000x reference)
"""AGDN (2-layer, K=3 hop) GNN message passing on 8 TRN2 NeuronCores.

Strategy (node-sharded, replicated feature table):
  - Nodes padded to 50176 = 8 * 6272; core c owns dst rows [c*6272, (c+1)*6272).
  - Per hop, each core processes the ~100K edges whose dst is in its block:
    SWDGE dma_gather pieces pull f[src] from the core's HBM copy of the
    full feature table (layer 1: f32 rows, 256B elems, int16 indices made
    valid by splitting each piece into two gathers over the two table
    halves; layer 2: bf16 node-pair 256B elems with parity-sliced rhs).
    PE segment-matmuls (one-hot S built on-chip by DVE is_equal vs iota)
    accumulate messages into per-128-dst-window PSUM tiles; windows are
    chunk-consecutive so each window drains once via ACT into the local
    f32 hop stack; an 8-core AllGather rebuilds the replicated table.
  - Dense layers, hop-attention softmax and ELU run locally per shard.
  - All edge sorting / chunk scheduling happens on host in numpy; the
    schedule is shared by all 8 cores (one SPMD graph), padded per
    (window, half/parity) group to the max count over cores.
"""

import numpy as np
import ml_dtypes

import concourse.bacc as bacc
import concourse.mybir as mybir
import concourse.tile as tile
from concourse.bass_utils import run_bass_kernel_spmd

N = 50000
E = 800000
DIN = 128
D = 64
NCORES = 8
P = 128
WPC = 49                  # 128-dst windows per core
NB = WPC * P              # 6272 nodes per core
NPAD = NCORES * NB        # 50176
NHALF = NPAD // 2
NG = WPC * 2
PIECE = 32                # gather chunks per piece
SBATCH = 8                # chunks per DVE S-build op
SLOPE = 0.2

F32 = mybir.dt.float32
BF16 = mybir.dt.bfloat16
I16 = mybir.dt.int16
BF = ml_dtypes.bfloat16


def _schedule(src_key, wloc, dstw, core, m_idx, split_keys=True):
    """Build one layer's schedule.

    src_key: 0/1 per edge (table half for L1, pair parity for L2)
    m_idx:   int16-safe gather index per edge
    Returns (CH, chunk_meta, pieces, idx_all, dstw_all) where chunk_meta
    entries are (window, key, first, last, slotcol).
    """
    g_in_core = wloc * 2 + src_key
    gkey = core * NG + g_in_core
    order = np.argsort(gkey, kind="stable")
    gk_sorted = gkey[order]
    run_start = np.searchsorted(gk_sorted, gk_sorted)
    pos_in_group = np.arange(E) - run_start
    inv = np.empty(E, dtype=np.int64)
    inv[order] = np.arange(E)
    pos_of_edge = pos_in_group[inv]

    cnt = np.bincount(gkey, minlength=NCORES * NG).reshape(NCORES, NG)
    gmax = cnt.max(axis=0)
    nchunks = np.ceil(gmax / P).astype(np.int64)   # per (w,key): g = 2w+key
    for w in range(WPC):
        if nchunks[2 * w] + nchunks[2 * w + 1] == 0:
            nchunks[2 * w] = 1
    CH = int(nchunks.sum())

    # program order: window-major, key-minor
    prog = []              # (w, key) per chunk
    first_chunk_of_group = {}
    for w in range(WPC):
        for key in (0, 1):
            if nchunks[2 * w + key]:
                first_chunk_of_group[(w, key)] = len(prog)
            for _ in range(int(nchunks[2 * w + key])):
                prog.append((w, key))

    # pieces + slot column assignment (key-0 chunks first within piece)
    cuts = []
    c0 = 0
    while c0 < CH:
        cuts.append((c0, min(c0 + PIECE, CH)))
        c0 += PIECE
    if len(cuts) > 1:
        lo, hi = cuts[-1]
        cuts = cuts[:-1]
        n = hi - lo
        if n > 16:
            cuts += [(lo, lo + n - 16), (lo + n - 16, lo + n - 8),
                     (lo + n - 8, hi)]
        elif n > 8:
            cuts += [(lo, lo + n - 8), (lo + n - 8, hi)]
        else:
            cuts.append((lo, hi))
    pieces = []            # (col0, [(key, ncols)...])
    col_of_chunk = [0] * CH
    for (c0, c1) in cuts:
        if not split_keys:
            for c in range(c0, c1):
                col_of_chunk[c] = c
            pieces.append((c0, [(0, c1 - c0)]))
            continue
        ids0 = [c for c in range(c0, c1) if prog[c][1] == 0]
        ids1 = [c for c in range(c0, c1) if prog[c][1] == 1]
        col = c0
        subs = []
        for key, ids in ((0, ids0), (1, ids1)):
            if ids:
                subs.append((key, len(ids)))
                for c in ids:
                    col_of_chunk[c] = col
                    col += 1
        pieces.append((c0, subs))

    # chunk meta with per-window first/last accumulation flags
    wtot = [0] * WPC
    for w, key in prog:
        wtot[w] += 1
    wseen = [0] * WPC
    chunk_meta = []
    for c, (w, key) in enumerate(prog):
        chunk_meta.append((w, key, wseen[w] == 0, wseen[w] == wtot[w] - 1,
                           col_of_chunk[c]))
        wseen[w] += 1

    # per-core slot arrays
    gfirst = np.zeros(NG, dtype=np.int64)
    for (w, key), fc in first_chunk_of_group.items():
        gfirst[2 * w + key] = fc
    col_arr = np.asarray(col_of_chunk, dtype=np.int64)
    idx_all, dstw_all = [], []
    nslots = CH * P
    for c in range(NCORES):
        mask = core == c
        g = g_in_core[mask]
        pos = pos_of_edge[mask]
        ch = gfirst[g] + (pos >> 7)
        slot = col_arr[ch] * P + (pos & 127)
        iarr = np.zeros(nslots, dtype=np.int16)
        darr = np.full(nslots, -1.0, dtype=np.float32)
        iarr[slot] = m_idx[mask].astype(np.int16)
        darr[slot] = dstw[mask].astype(np.float32)
        wrapped = np.tile(iarr.reshape(CH * 8, 16).T, (8, 1))
        idx_all.append(np.ascontiguousarray(wrapped))
        dstw_all.append(np.ascontiguousarray(
            darr.reshape(CH, P).T.astype(BF)))
    return CH, chunk_meta, pieces, idx_all, dstw_all


def _preprocess(edge_index):
    src = np.ascontiguousarray(edge_index[0]).astype(np.int64)
    dst = np.ascontiguousarray(edge_index[1]).astype(np.int64)
    core = dst // NB
    wloc = (dst % NB) >> 7
    dstw = dst & 127
    sc = src // NB
    sr = src % NB
    m_src = sc * NB + (sr & 127) * WPC + (sr >> 7)   # device node id

    half = (m_src >= NHALF).astype(np.int64)
    s1 = _schedule(half, wloc, dstw, core, m_src - half * NHALF)
    par = m_src & 1
    s2 = _schedule(par, wloc, dstw, core, m_src >> 1, split_keys=False)
    return s1, s2


def _build(s1_struct, s2_struct):
    CH1, meta1, pieces1, _, _ = s1_struct
    CH2, meta2, pieces2, _, _ = s2_struct
    CHmax = max(CH1, CH2)

    nc = bacc.Bacc("TRN2", target_bir_lowering=False, debug=False,
                   num_devices=NCORES, num_swdge_queues=4)

    with tile.TileContext(nc) as tc:
        xT = nc.dram_tensor("xT", [WPC, P, P], F32, kind="ExternalInput")
        idxp1 = nc.dram_tensor("idxp1", [P, CH1 * 8], I16, kind="ExternalInput")
        dstwp1 = nc.dram_tensor("dstwp1", [P, CH1], BF16, kind="ExternalInput")
        idxp2 = nc.dram_tensor("idxp2", [P, CH2 * 8], I16, kind="ExternalInput")
        dstwp2 = nc.dram_tensor("dstwp2", [P, CH2], BF16, kind="ExternalInput")
        iotap = nc.dram_tensor("iotap", [P, P], BF16, kind="ExternalInput")
        identp = nc.dram_tensor("identp", [P, P], F32, kind="ExternalInput")
        w1tp = nc.dram_tensor("w1tp", [P, D], F32, kind="ExternalInput")
        rw1tp = nc.dram_tensor("rw1tp", [P, D], F32, kind="ExternalInput")
        w2tp = nc.dram_tensor("w2tp", [D, D], F32, kind="ExternalInput")
        attp = nc.dram_tensor("attp", [P, 4 * D], F32, kind="ExternalInput")
        biasp = nc.dram_tensor("biasp", [P, 2 * D], F32, kind="ExternalInput")
        outp = nc.dram_tensor("out", [NB, D], F32, kind="ExternalOutput")

        cc1 = [nc.dram_tensor(f"ccA{i}", [NB, D], F32) for i in range(3)]
        tabs1 = [nc.dram_tensor(f"tabA{i}", [NPAD, D], F32,
                                addr_space="Shared") for i in range(3)]
        cc2 = [nc.dram_tensor(f"ccB{i}", [NB, D], BF16) for i in range(3)]
        tabs2 = [nc.dram_tensor(f"tabB{i}", [NPAD, D], BF16,
                                addr_space="Shared") for i in range(3)]

        with tc.tile_pool(name="const", bufs=1) as pconst, \
             tc.tile_pool(name="idxpool", bufs=1) as pidx, \
             tc.tile_pool(name="big", bufs=1) as pbig, \
             tc.tile_pool(name="xt", bufs=3) as pxt, \
             tc.tile_pool(name="idxt", bufs=4) as pidx2, \
             tc.tile_pool(name="outl", bufs=1) as pout, \
             tc.tile_pool(name="g2", bufs=4) as pg2, \
             tc.tile_pool(name="sseg", bufs=3) as pseg, \
             tc.tile_pool(name="att", bufs=1) as patt, \
             tc.tile_pool(name="psum", bufs=4, space="PSUM") as pps, \
             tc.tile_pool(name="psumd", bufs=2, space="PSUM") as ppsd:

            iota_sb = pconst.tile([P, P], BF16)
            nc.sync.dma_start(out=iota_sb[:], in_=iotap[:])
            ident_sb = pconst.tile([P, P], F32)
            nc.sync.dma_start(out=ident_sb[:], in_=identp[:])
            w1t_sb = pconst.tile([P, D], F32)
            nc.sync.dma_start(out=w1t_sb[:], in_=w1tp[:])
            rw1t_sb = pconst.tile([P, D], F32)
            nc.sync.dma_start(out=rw1t_sb[:], in_=rw1tp[:])
            w2t_sb = pconst.tile([D, D], F32)
            nc.sync.dma_start(out=w2t_sb[:], in_=w2tp[:])
            att_sb = pconst.tile([P, 4 * D], F32)
            nc.sync.dma_start(out=att_sb[:], in_=attp[:])
            bias_sb = pconst.tile([P, 2 * D], F32)
            nc.sync.dma_start(out=bias_sb[:], in_=biasp[:])

            dstw_sb = pidx.tile([P, CHmax], BF16)
            nc.sync.dma_start(out=dstw_sb[:, 0:CH1], in_=dstwp1[:])

            stack1 = pbig.tile([P, WPC, 4, D], F32)
            stack2 = pbig.tile([P, WPC, 4, D], F32)
            res1_sb = pbig.tile([P, WPC, D], F32)
            gT_sb = pbig.tile([D, WPC, P], F32)
            g_sb = pout.tile([P, WPC, D], F32, tag="outl", name="g_sb")

            def dense_layer1():
                for t in range(WPC):
                    xtile = pxt.tile([P, P], F32, tag="xt", name="xtile")
                    nc.sync.dma_start(out=xtile[:], in_=xT[t, :, :])
                    ps = ppsd.tile([P, D], F32, tag="pd")
                    nc.tensor.matmul(out=ps[:], lhsT=xtile[:], rhs=w1t_sb[:],
                                     start=True, stop=True)
                    nc.vector.tensor_copy(out=stack1[:, t, 0, :], in_=ps[:])
                    ps2 = ppsd.tile([P, D], F32, tag="pd")
                    nc.tensor.matmul(out=ps2[:], lhsT=xtile[:], rhs=rw1t_sb[:],
                                     start=True, stop=True)
                    nc.scalar.copy(out=res1_sb[:, t, :], in_=ps2[:])

            def do_allgather(cc, tab, stack, k, f32):
                eng = nc.sync if f32 else nc.gpsimd   # bf16 needs cast DMA
                eng.dma_start(
                    out=cc[:].rearrange("(p w) d -> p w d", p=P),
                    in_=stack[:, :, k, :],
                )
                nc.gpsimd.collective_compute(
                    "AllGather", mybir.AluOpType.bypass,
                    replica_groups=[list(range(NCORES))],
                    ins=[cc[:].opt()],
                    outs=[tab[:].opt()],
                )

            qctr = [0]

            def hop(layer, tab, stack, k):
                """stack[:, :, k, :] = A @ table."""
                if layer == 1:
                    meta, pieces, sdt, gcols = meta1, pieces1, F32, D
                    idxp = idxp1
                    bases = [tab[0:NHALF, :], tab[NHALF:NPAD, :]]
                else:
                    meta, pieces, sdt, gcols = meta2, pieces2, BF16, 2 * D
                    idxp = idxp2
                    bases = [tab[:].rearrange("(a b) d -> a (b d)", b=2)] * 2
                pw = None
                for (pc0, subs) in pieces:
                    npc = sum(n for _, n in subs)
                    g2 = pg2.tile([P, PIECE, gcols], sdt if gcols == 2 * D else F32, tag="g2", name="g2")
                    idxt = pidx2.tile([P, PIECE * 8], I16, tag="idxt",
                                      name="idxt")
                    nc.sync.dma_start(out=idxt[:, 0:npc * 8],
                                      in_=idxp[:, pc0 * 8:(pc0 + npc) * 8])
                    scol = 0
                    for key, ncol in subs:
                        nc.gpsimd.dma_gather(
                            g2[:, scol:scol + ncol, :], bases[key],
                            idxt[:, scol * 8:(scol + ncol) * 8],
                            ncol * P, ncol * P, gcols,
                            single_packet=False, queue_num=qctr[0] % 4,
                        )
                        qctr[0] += 1
                        scol += ncol
                    sb_tiles = {}
                    for b0 in range(0, npc, SBATCH):
                        b1 = min(b0 + SBATCH, npc)
                        st = pseg.tile([P, SBATCH * P], sdt, tag="sseg",
                                       name="st")
                        nc.vector.tensor_tensor(
                            out=st[:, 0:(b1 - b0) * P].rearrange(
                                "p (c q) -> p c q", q=P),
                            in0=dstw_sb[:, pc0 + b0:pc0 + b1].to_broadcast(
                                [P, b1 - b0, P]),
                            in1=iota_sb[:].unsqueeze(1).broadcast_to(
                                [P, b1 - b0, P]),
                            op=mybir.AluOpType.is_equal,
                        )
                        sb_tiles[b0] = st
                    for c in range(pc0, pc0 + npc):
                        w, key, first, last, col = meta[c]
                        if first:
                            pw = pps.tile([P, D], F32, tag="pwin", name="pw")
                        lcol = col - pc0
                        st = sb_tiles[(lcol // SBATCH) * SBATCH]
                        j = lcol % SBATCH
                        if layer == 1:
                            rhs = g2[:, lcol, :]
                        else:
                            rhs = g2[:, lcol, key * D:(key + 1) * D]
                        nc.tensor.matmul(
                            out=pw[:], lhsT=st[:, j * P:(j + 1) * P],
                            rhs=rhs, start=first, stop=last,
                        )
                        if last:
                            nc.scalar.copy(out=stack[:, w, k, :], in_=pw[:])

            def attention_early(stack, scratch, bias_col, sm):
                """Score terms that only need hops 0..2 (run before hop 3)."""
                a_hop = att_sb[:, (2 * bias_col + 1) * D:(2 * bias_col + 2) * D]
                a_h0 = att_sb[:, (2 * bias_col) * D:(2 * bias_col + 1) * D]
                tmp = scratch[:, :, 0, :]
                scf = sm[:, :, 0:4]
                sc0 = sm[:, :, 4:5]
                for k in range(3):
                    nc.vector.tensor_tensor(
                        out=tmp, in0=stack[:, :, k, :],
                        in1=a_hop.unsqueeze(1).broadcast_to([P, WPC, D]),
                        op=mybir.AluOpType.mult)
                    nc.vector.reduce_sum(out=scf[:, :, k:k + 1], in_=tmp,
                                         axis=mybir.AxisListType.X)
                nc.vector.tensor_tensor(
                    out=tmp, in0=stack[:, :, 0, :],
                    in1=a_h0.unsqueeze(1).broadcast_to([P, WPC, D]),
                    op=mybir.AluOpType.mult)
                nc.vector.reduce_sum(out=sc0, in_=tmp,
                                     axis=mybir.AxisListType.X)

            def attention(stack, scratch, res_ap, bias_col, out_tile, sm):
                a_hop = att_sb[:, (2 * bias_col + 1) * D:(2 * bias_col + 2) * D]
                tmp = scratch[:, :, 0, :]
                acc0 = scratch[:, :, 1, :]
                scf = sm[:, :, 0:4]
                scores = sm[:, :, 0:4]
                sc0 = sm[:, :, 4:5]
                mx = sm[:, :, 5:6]
                ssum = sm[:, :, 6:7]
                rec = sm[:, :, 7:8]
                nc.vector.tensor_tensor(
                    out=tmp, in0=stack[:, :, 3, :],
                    in1=a_hop.unsqueeze(1).broadcast_to([P, WPC, D]),
                    op=mybir.AluOpType.mult)
                nc.vector.reduce_sum(out=scf[:, :, 3:4], in_=tmp,
                                     axis=mybir.AxisListType.X)
                nc.vector.tensor_tensor(
                    out=scores, in0=scf, in1=sc0.broadcast_to([P, WPC, 4]),
                    op=mybir.AluOpType.add)
                nc.vector.scalar_tensor_tensor(
                    out=scores, in0=scores, scalar=SLOPE, in1=scores,
                    op0=mybir.AluOpType.mult, op1=mybir.AluOpType.max)
                nc.vector.reduce_max(out=mx, in_=scores,
                                     axis=mybir.AxisListType.X)
                nc.vector.tensor_tensor(
                    out=scores, in0=scores, in1=mx.broadcast_to([P, WPC, 4]),
                    op=mybir.AluOpType.subtract)
                nc.scalar.activation(out=scores, in_=scores,
                                     func=mybir.ActivationFunctionType.Exp)
                nc.vector.reduce_sum(out=ssum, in_=scores,
                                     axis=mybir.AxisListType.X)
                nc.vector.reciprocal(out=rec, in_=ssum)
                nc.vector.tensor_tensor(
                    out=scores, in0=scores, in1=rec.broadcast_to([P, WPC, 4]),
                    op=mybir.AluOpType.mult)
                nc.vector.tensor_tensor(
                    out=acc0, in0=stack[:, :, 0, :],
                    in1=scores[:, :, 0:1].broadcast_to([P, WPC, D]),
                    op=mybir.AluOpType.mult)
                for k in range(1, 4):
                    nc.vector.tensor_tensor(
                        out=tmp, in0=stack[:, :, k, :],
                        in1=scores[:, :, k:k + 1].broadcast_to([P, WPC, D]),
                        op=mybir.AluOpType.mult)
                    nc.vector.tensor_tensor(out=acc0, in0=acc0, in1=tmp,
                                            op=mybir.AluOpType.add)
                nc.vector.tensor_tensor(out=acc0, in0=acc0, in1=res_ap,
                                        op=mybir.AluOpType.add)
                b = bias_sb[:, bias_col * D:(bias_col + 1) * D]
                nc.vector.tensor_tensor(
                    out=out_tile[:], in0=acc0,
                    in1=b.unsqueeze(1).broadcast_to([P, WPC, D]),
                    op=mybir.AluOpType.add)

            # ---------------- layer 1 ----------------
            dense_layer1()
            do_allgather(cc1[0], tabs1[0], stack1, 0, True)
            hop(1, tabs1[0], stack1, 1)
            do_allgather(cc1[1], tabs1[1], stack1, 1, True)
            hop(1, tabs1[1], stack1, 2)
            do_allgather(cc1[2], tabs1[2], stack1, 2, True)
            sm1 = patt.tile([P, WPC, 8], F32, tag="attsm", name="sm1")
            attention_early(stack1, stack2, 0, sm1)
            hop(1, tabs1[2], stack1, 3)

            # swap dstw schedule for layer 2 (overlaps with attention)
            nc.sync.dma_start(out=dstw_sb[:, 0:CH2], in_=dstwp2[:])

            attention(stack1, stack2, res1_sb[:], 0, g_sb, sm1)
            gm = stack2[:, :, 2, :]
            nc.vector.tensor_scalar_min(out=gm, in0=g_sb[:], scalar1=0.0)
            nc.scalar.activation(out=gm, in_=gm,
                                 func=mybir.ActivationFunctionType.Exp)
            gp = stack2[:, :, 3, :]
            nc.vector.tensor_scalar_max(out=gp, in0=g_sb[:], scalar1=0.0)
            nc.vector.scalar_tensor_tensor(
                out=g_sb[:], in0=gm, scalar=-1.0, in1=gp,
                op0=mybir.AluOpType.add, op1=mybir.AluOpType.add)

            # ---------------- layer 2 ----------------
            for t in range(WPC):
                pst = ppsd.tile([D, P], F32, tag="pd", name="pst")
                nc.tensor.transpose(out=pst[:], in_=g_sb[:, t, :],
                                    identity=ident_sb[:])
                nc.vector.tensor_copy(out=gT_sb[:, t, :], in_=pst[:])
            for t in range(WPC):
                ps = ppsd.tile([P, D], F32, tag="pd", name="ps2")
                nc.tensor.matmul(out=ps[:], lhsT=gT_sb[:, t, :],
                                 rhs=w2t_sb[:], start=True, stop=True)
                nc.vector.tensor_copy(out=stack2[:, t, 0, :], in_=ps[:])
            do_allgather(cc2[0], tabs2[0], stack2, 0, False)
            hop(2, tabs2[0], stack2, 1)
            do_allgather(cc2[1], tabs2[1], stack2, 1, False)
            hop(2, tabs2[1], stack2, 2)
            do_allgather(cc2[2], tabs2[2], stack2, 2, False)
            sm2 = patt.tile([P, WPC, 8], F32, tag="attsm", name="sm2")
            attention_early(stack2, stack1, 1, sm2)
            hop(2, tabs2[2], stack2, 3)

            out2_sb = pout.tile([P, WPC, D], F32, tag="outl", name="out2_sb")
            attention(stack2, stack1, stack2[:, :, 0, :], 1, out2_sb, sm2)
            nc.sync.dma_start(
                out=outp[:].rearrange("(p w) d -> p w d", p=P),
                in_=out2_sb[:],
            )
    nc.compile()
    return nc


_CACHE = {}
_last_in_maps = None


def kernel(**inputs):
    x = np.asarray(inputs["x"], dtype=np.float32)
    edge_index = np.asarray(inputs["edge_index"])
    W1 = np.asarray(inputs["W1"], dtype=np.float32)
    att1 = np.asarray(inputs["att1"], dtype=np.float32)
    bias1 = np.asarray(inputs["bias1"], dtype=np.float32)
    resW1 = np.asarray(inputs["resW1"], dtype=np.float32)
    W2 = np.asarray(inputs["W2"], dtype=np.float32)
    att2 = np.asarray(inputs["att2"], dtype=np.float32)
    bias2 = np.asarray(inputs["bias2"], dtype=np.float32)

    s1, s2 = _preprocess(edge_index)
    CH1, CH2 = s1[0], s2[0]

    key = ("k", CH1, CH2, tuple(s1[1][:4]), tuple(s2[1][:4]))
    if key not in _CACHE:
        _CACHE[key] = _build(s1, s2)
    nc = _CACHE[key]

    xpad = np.zeros((NPAD, DIN), dtype=np.float32)
    xpad[:N] = x
    iota_np = np.tile(np.arange(P, dtype=np.float32)[None, :], (P, 1)).astype(BF)
    ident_np = np.eye(P, dtype=np.float32)
    att_np = np.concatenate([
        np.tile(att1[0, 0, :D][None, :], (P, 1)),
        np.tile(att1[0, 0, D:][None, :], (P, 1)),
        np.tile(att2[0, 0, :D][None, :], (P, 1)),
        np.tile(att2[0, 0, D:][None, :], (P, 1)),
    ], axis=1).astype(np.float32)
    bias_np = np.concatenate([
        np.tile(bias1[None, :], (P, 1)),
        np.tile(bias2[None, :], (P, 1)),
    ], axis=1).astype(np.float32)
    w1t = np.ascontiguousarray(W1.T)
    rw1t = np.ascontiguousarray(resW1.T)
    w2t = np.ascontiguousarray(W2.T)

    jj = np.arange(NB)
    real_in_block = (jj % WPC) * P + (jj // WPC)

    in_maps = []
    for c in range(NCORES):
        xT_c = np.ascontiguousarray(
            xpad[c * NB:(c + 1) * NB].T.reshape(P, WPC, P).transpose(1, 0, 2))
        in_maps.append({
            "xT": xT_c,
            "idxp1": s1[3][c], "dstwp1": s1[4][c],
            "idxp2": s2[3][c], "dstwp2": s2[4][c],
            "iotap": iota_np, "identp": ident_np,
            "w1tp": w1t, "rw1tp": rw1t, "w2tp": w2t,
            "attp": att_np, "biasp": bias_np,
        })

    global _last_in_maps
    _last_in_maps = in_maps
    res = run_bass_kernel_spmd(nc, in_maps, core_ids=list(range(NCORES)))
    out = np.empty((NPAD, D), dtype=np.float32)
    for c in range(NCORES):
        out[c * NB + real_in_block] = res.results[c]["out"]
    return out[:N].astype(np.float32)


# revision 37
# speedup vs baseline: 1.0480x; 1.0480x over previous
"""AGDN (2-layer, K=3 hop) GNN message passing on 8 TRN2 NeuronCores.

Strategy (node-sharded, replicated feature table):
  - Nodes padded to 50176 = 8 * 6272; core c owns dst rows [c*6272, (c+1)*6272).
  - Per hop, each core processes the ~100K edges whose dst is in its block:
    SWDGE dma_gather pieces pull f[src] from the core's HBM copy of the
    full feature table (layer 1: f32 rows, 256B elems, int16 indices made
    valid by splitting each piece into two gathers over the two table
    halves; layer 2: bf16 node-pair 256B elems with parity-sliced rhs).
    PE segment-matmuls (one-hot S built on-chip by DVE is_equal vs iota)
    accumulate messages into per-128-dst-window PSUM tiles; windows are
    chunk-consecutive so each window drains once via ACT into the local
    f32 hop stack; an 8-core AllGather rebuilds the replicated table.
  - Dense layers, hop-attention softmax and ELU run locally per shard.
  - All edge sorting / chunk scheduling happens on host in numpy; the
    schedule is shared by all 8 cores (one SPMD graph), padded per
    (window, half/parity) group to the max count over cores.
"""

import numpy as np
import ml_dtypes

import concourse.bacc as bacc
import concourse.mybir as mybir
import concourse.tile as tile
from concourse.bass_utils import run_bass_kernel_spmd

N = 50000
E = 800000
DIN = 128
D = 64
NCORES = 8
P = 128
WPC = 49                  # 128-dst windows per core
NB = WPC * P              # 6272 nodes per core
NPAD = NCORES * NB        # 50176
NHALF = NPAD // 2
NG = WPC * 2
PIECE = 32                # gather chunks per piece
SBATCH = 8                # chunks per DVE S-build op
SLOPE = 0.2

F32 = mybir.dt.float32
BF16 = mybir.dt.bfloat16
I16 = mybir.dt.int16
BF = ml_dtypes.bfloat16


def _schedule(src_key, wloc, dstw, core, m_idx, split_keys=True):
    """Build one layer's schedule.

    src_key: 0/1 per edge (table half for L1, pair parity for L2)
    m_idx:   int16-safe gather index per edge
    Returns (CH, chunk_meta, pieces, idx_all, dstw_all) where chunk_meta
    entries are (window, key, first, last, slotcol).
    """
    g_in_core = wloc * 2 + src_key
    gkey = core * NG + g_in_core
    order = np.argsort(gkey, kind="stable")
    gk_sorted = gkey[order]
    run_start = np.searchsorted(gk_sorted, gk_sorted)
    pos_in_group = np.arange(E) - run_start
    inv = np.empty(E, dtype=np.int64)
    inv[order] = np.arange(E)
    pos_of_edge = pos_in_group[inv]

    cnt = np.bincount(gkey, minlength=NCORES * NG).reshape(NCORES, NG)
    gmax = cnt.max(axis=0)
    nchunks = np.ceil(gmax / P).astype(np.int64)   # per (w,key): g = 2w+key
    for w in range(WPC):
        if nchunks[2 * w] + nchunks[2 * w + 1] == 0:
            nchunks[2 * w] = 1
    CH = int(nchunks.sum())

    # program order: window-major, key-minor
    prog = []              # (w, key) per chunk
    first_chunk_of_group = {}
    for w in range(WPC):
        for key in (0, 1):
            if nchunks[2 * w + key]:
                first_chunk_of_group[(w, key)] = len(prog)
            for _ in range(int(nchunks[2 * w + key])):
                prog.append((w, key))

    # pieces + slot column assignment (key-0 chunks first within piece)
    cuts = []
    c0 = 0
    while c0 < CH:
        cuts.append((c0, min(c0 + PIECE, CH)))
        c0 += PIECE
    if len(cuts) > 1:
        lo, hi = cuts[-1]
        cuts = cuts[:-1]
        n = hi - lo
        if n > 16:
            cuts += [(lo, lo + n - 16), (lo + n - 16, lo + n - 8),
                     (lo + n - 8, hi)]
        elif n > 8:
            cuts += [(lo, lo + n - 8), (lo + n - 8, hi)]
        else:
            cuts.append((lo, hi))
    pieces = []            # (col0, [(key, ncols)...])
    col_of_chunk = [0] * CH
    for (c0, c1) in cuts:
        if not split_keys:
            for c in range(c0, c1):
                col_of_chunk[c] = c
            pieces.append((c0, [(0, c1 - c0)]))
            continue
        ids0 = [c for c in range(c0, c1) if prog[c][1] == 0]
        ids1 = [c for c in range(c0, c1) if prog[c][1] == 1]
        col = c0
        subs = []
        for key, ids in ((0, ids0), (1, ids1)):
            if ids:
                subs.append((key, len(ids)))
                for c in ids:
                    col_of_chunk[c] = col
                    col += 1
        pieces.append((c0, subs))

    # chunk meta with per-window first/last accumulation flags
    wtot = [0] * WPC
    for w, key in prog:
        wtot[w] += 1
    wseen = [0] * WPC
    chunk_meta = []
    for c, (w, key) in enumerate(prog):
        chunk_meta.append((w, key, wseen[w] == 0, wseen[w] == wtot[w] - 1,
                           col_of_chunk[c]))
        wseen[w] += 1

    # per-core slot arrays
    gfirst = np.zeros(NG, dtype=np.int64)
    for (w, key), fc in first_chunk_of_group.items():
        gfirst[2 * w + key] = fc
    col_arr = np.asarray(col_of_chunk, dtype=np.int64)
    idx_all, dstw_all = [], []
    nslots = CH * P
    for c in range(NCORES):
        mask = core == c
        g = g_in_core[mask]
        pos = pos_of_edge[mask]
        ch = gfirst[g] + (pos >> 7)
        slot = col_arr[ch] * P + (pos & 127)
        iarr = np.zeros(nslots, dtype=np.int16)
        darr = np.full(nslots, -1.0, dtype=np.float32)
        iarr[slot] = m_idx[mask].astype(np.int16)
        darr[slot] = dstw[mask].astype(np.float32)
        wrapped = np.tile(iarr.reshape(CH * 8, 16).T, (8, 1))
        idx_all.append(np.ascontiguousarray(wrapped))
        dstw_all.append(np.ascontiguousarray(
            darr.reshape(CH, P).T.astype(BF)))
    return CH, chunk_meta, pieces, idx_all, dstw_all


def _preprocess(edge_index):
    src = np.ascontiguousarray(edge_index[0]).astype(np.int64)
    dst = np.ascontiguousarray(edge_index[1]).astype(np.int64)
    core = dst // NB
    wloc = (dst % NB) >> 7
    dstw = dst & 127
    sc = src // NB
    sr = src % NB
    m_src = sc * NB + (sr & 127) * WPC + (sr >> 7)   # device node id

    half = (m_src >= NHALF).astype(np.int64)
    s1 = _schedule(half, wloc, dstw, core, m_src - half * NHALF)
    par = m_src & 1
    s2 = _schedule(par, wloc, dstw, core, m_src >> 1)
    return s1, s2


def _build(s1_struct, s2_struct):
    CH1, meta1, pieces1, _, _ = s1_struct
    CH2, meta2, pieces2, _, _ = s2_struct
    CHmax = max(CH1, CH2)

    nc = bacc.Bacc("TRN2", target_bir_lowering=False, debug=False,
                   num_devices=NCORES, num_swdge_queues=4)

    with tile.TileContext(nc) as tc:
        xT = nc.dram_tensor("xT", [WPC, P, P], F32, kind="ExternalInput")
        idxp1 = nc.dram_tensor("idxp1", [P, CH1 * 8], I16, kind="ExternalInput")
        dstwp1 = nc.dram_tensor("dstwp1", [P, CH1], BF16, kind="ExternalInput")
        idxp2 = nc.dram_tensor("idxp2", [P, CH2 * 8], I16, kind="ExternalInput")
        dstwp2 = nc.dram_tensor("dstwp2", [P, CH2], BF16, kind="ExternalInput")
        iotap = nc.dram_tensor("iotap", [P, P], BF16, kind="ExternalInput")
        identp = nc.dram_tensor("identp", [P, P], F32, kind="ExternalInput")
        w1tp = nc.dram_tensor("w1tp", [P, D], F32, kind="ExternalInput")
        rw1tp = nc.dram_tensor("rw1tp", [P, D], F32, kind="ExternalInput")
        w2tp = nc.dram_tensor("w2tp", [D, D], F32, kind="ExternalInput")
        attp = nc.dram_tensor("attp", [P, 4 * D], F32, kind="ExternalInput")
        biasp = nc.dram_tensor("biasp", [P, 2 * D], F32, kind="ExternalInput")
        outp = nc.dram_tensor("out", [NB, D], F32, kind="ExternalOutput")

        cc1 = [nc.dram_tensor(f"ccA{i}", [NB, D], F32) for i in range(3)]
        tabs1 = [nc.dram_tensor(f"tabA{i}", [NPAD, D], F32,
                                addr_space="Shared") for i in range(3)]
        cc2 = [nc.dram_tensor(f"ccB{i}", [NB, D], BF16) for i in range(3)]
        tabs2 = [nc.dram_tensor(f"tabB{i}", [NPAD, D], BF16,
                                addr_space="Shared") for i in range(3)]

        with tc.tile_pool(name="const", bufs=1) as pconst, \
             tc.tile_pool(name="idxpool", bufs=1) as pidx, \
             tc.tile_pool(name="big", bufs=1) as pbig, \
             tc.tile_pool(name="xt", bufs=3) as pxt, \
             tc.tile_pool(name="idxt", bufs=4) as pidx2, \
             tc.tile_pool(name="outl", bufs=1) as pout, \
             tc.tile_pool(name="g2", bufs=4) as pg2, \
             tc.tile_pool(name="sseg", bufs=3) as pseg, \
             tc.tile_pool(name="att", bufs=1) as patt, \
             tc.tile_pool(name="psum", bufs=4, space="PSUM") as pps, \
             tc.tile_pool(name="psumd", bufs=2, space="PSUM") as ppsd:

            iota_sb = pconst.tile([P, P], BF16)
            nc.sync.dma_start(out=iota_sb[:], in_=iotap[:])
            ident_sb = pconst.tile([P, P], F32)
            nc.sync.dma_start(out=ident_sb[:], in_=identp[:])
            w1t_sb = pconst.tile([P, D], F32)
            nc.sync.dma_start(out=w1t_sb[:], in_=w1tp[:])
            rw1t_sb = pconst.tile([P, D], F32)
            nc.sync.dma_start(out=rw1t_sb[:], in_=rw1tp[:])
            w2t_sb = pconst.tile([D, D], F32)
            nc.sync.dma_start(out=w2t_sb[:], in_=w2tp[:])
            att_sb = pconst.tile([P, 4 * D], F32)
            nc.sync.dma_start(out=att_sb[:], in_=attp[:])
            bias_sb = pconst.tile([P, 2 * D], F32)
            nc.sync.dma_start(out=bias_sb[:], in_=biasp[:])

            dstw_sb = pidx.tile([P, CHmax], BF16)
            nc.sync.dma_start(out=dstw_sb[:, 0:CH1], in_=dstwp1[:])

            stack1 = pbig.tile([P, WPC, 4, D], F32)
            stack2 = pbig.tile([P, WPC, 4, D], F32)
            res1_sb = pbig.tile([P, WPC, D], F32)
            gT_sb = pbig.tile([D, WPC, P], F32)
            g_sb = pout.tile([P, WPC, D], F32, tag="outl", name="g_sb")

            def dense_layer1():
                for t in range(WPC):
                    xtile = pxt.tile([P, P], F32, tag="xt", name="xtile")
                    nc.sync.dma_start(out=xtile[:], in_=xT[t, :, :])
                    ps = ppsd.tile([P, D], F32, tag="pd")
                    nc.tensor.matmul(out=ps[:], lhsT=xtile[:], rhs=w1t_sb[:],
                                     start=True, stop=True)
                    nc.vector.tensor_copy(out=stack1[:, t, 0, :], in_=ps[:])
                    ps2 = ppsd.tile([P, D], F32, tag="pd")
                    nc.tensor.matmul(out=ps2[:], lhsT=xtile[:], rhs=rw1t_sb[:],
                                     start=True, stop=True)
                    nc.scalar.copy(out=res1_sb[:, t, :], in_=ps2[:])

            def do_allgather(cc, tab, stack, k, f32):
                eng = nc.sync if f32 else nc.gpsimd   # bf16 needs cast DMA
                eng.dma_start(
                    out=cc[:].rearrange("(p w) d -> p w d", p=P),
                    in_=stack[:, :, k, :],
                )
                nc.gpsimd.collective_compute(
                    "AllGather", mybir.AluOpType.bypass,
                    replica_groups=[list(range(NCORES))],
                    ins=[cc[:].opt()],
                    outs=[tab[:].opt()],
                )

            qctr = [0]

            def hop(layer, tab, stack, k):
                """stack[:, :, k, :] = A @ table."""
                if layer == 1:
                    meta, pieces, sdt, gcols = meta1, pieces1, F32, D
                    idxp = idxp1
                    bases = [tab[0:NHALF, :], tab[NHALF:NPAD, :]]
                else:
                    meta, pieces, sdt, gcols = meta2, pieces2, BF16, 2 * D
                    idxp = idxp2
                    bases = [tab[:].rearrange("(a b) d -> a (b d)", b=2)] * 2
                pw = None
                for (pc0, subs) in pieces:
                    npc = sum(n for _, n in subs)
                    g2 = pg2.tile([P, PIECE, gcols], sdt if gcols == 2 * D else F32, tag="g2", name="g2")
                    idxt = pidx2.tile([P, PIECE * 8], I16, tag="idxt",
                                      name="idxt")
                    nc.sync.dma_start(out=idxt[:, 0:npc * 8],
                                      in_=idxp[:, pc0 * 8:(pc0 + npc) * 8])
                    scol = 0
                    for key, ncol in subs:
                        nc.gpsimd.dma_gather(
                            g2[:, scol:scol + ncol, :], bases[key],
                            idxt[:, scol * 8:(scol + ncol) * 8],
                            ncol * P, ncol * P, gcols,
                            single_packet=False, queue_num=qctr[0] % 4,
                        )
                        qctr[0] += 1
                        scol += ncol
                    sb_tiles = {}
                    for b0 in range(0, npc, SBATCH):
                        b1 = min(b0 + SBATCH, npc)
                        st = pseg.tile([P, SBATCH * P], sdt, tag="sseg",
                                       name="st")
                        nc.vector.tensor_tensor(
                            out=st[:, 0:(b1 - b0) * P].rearrange(
                                "p (c q) -> p c q", q=P),
                            in0=dstw_sb[:, pc0 + b0:pc0 + b1].to_broadcast(
                                [P, b1 - b0, P]),
                            in1=iota_sb[:].unsqueeze(1).broadcast_to(
                                [P, b1 - b0, P]),
                            op=mybir.AluOpType.is_equal,
                        )
                        sb_tiles[b0] = st
                    for c in range(pc0, pc0 + npc):
                        w, key, first, last, col = meta[c]
                        if first:
                            pw = pps.tile([P, D], F32, tag="pwin", name="pw")
                        lcol = col - pc0
                        st = sb_tiles[(lcol // SBATCH) * SBATCH]
                        j = lcol % SBATCH
                        if layer == 1:
                            rhs = g2[:, lcol, :]
                        else:
                            rhs = g2[:, lcol, key * D:(key + 1) * D]
                        nc.tensor.matmul(
                            out=pw[:], lhsT=st[:, j * P:(j + 1) * P],
                            rhs=rhs, start=first, stop=last,
                        )
                        if last:
                            nc.scalar.copy(out=stack[:, w, k, :], in_=pw[:])

            def attention_early(stack, scratch, bias_col, sm):
                """Score terms that only need hops 0..2 (run before hop 3)."""
                a_hop = att_sb[:, (2 * bias_col + 1) * D:(2 * bias_col + 2) * D]
                a_h0 = att_sb[:, (2 * bias_col) * D:(2 * bias_col + 1) * D]
                tmp = scratch[:, :, 0, :]
                scf = sm[:, :, 0:4]
                sc0 = sm[:, :, 4:5]
                for k in range(3):
                    nc.vector.tensor_tensor(
                        out=tmp, in0=stack[:, :, k, :],
                        in1=a_hop.unsqueeze(1).broadcast_to([P, WPC, D]),
                        op=mybir.AluOpType.mult)
                    nc.vector.reduce_sum(out=scf[:, :, k:k + 1], in_=tmp,
                                         axis=mybir.AxisListType.X)
                nc.vector.tensor_tensor(
                    out=tmp, in0=stack[:, :, 0, :],
                    in1=a_h0.unsqueeze(1).broadcast_to([P, WPC, D]),
                    op=mybir.AluOpType.mult)
                nc.vector.reduce_sum(out=sc0, in_=tmp,
                                     axis=mybir.AxisListType.X)

            def attention(stack, scratch, res_ap, bias_col, out_tile, sm):
                a_hop = att_sb[:, (2 * bias_col + 1) * D:(2 * bias_col + 2) * D]
                tmp = scratch[:, :, 0, :]
                acc0 = scratch[:, :, 1, :]
                scf = sm[:, :, 0:4]
                scores = sm[:, :, 0:4]
                sc0 = sm[:, :, 4:5]
                mx = sm[:, :, 5:6]
                ssum = sm[:, :, 6:7]
                rec = sm[:, :, 7:8]
                nc.vector.tensor_tensor(
                    out=tmp, in0=stack[:, :, 3, :],
                    in1=a_hop.unsqueeze(1).broadcast_to([P, WPC, D]),
                    op=mybir.AluOpType.mult)
                nc.vector.reduce_sum(out=scf[:, :, 3:4], in_=tmp,
                                     axis=mybir.AxisListType.X)
                nc.vector.tensor_tensor(
                    out=scores, in0=scf, in1=sc0.broadcast_to([P, WPC, 4]),
                    op=mybir.AluOpType.add)
                nc.vector.scalar_tensor_tensor(
                    out=scores, in0=scores, scalar=SLOPE, in1=scores,
                    op0=mybir.AluOpType.mult, op1=mybir.AluOpType.max)
                nc.vector.reduce_max(out=mx, in_=scores,
                                     axis=mybir.AxisListType.X)
                nc.vector.tensor_tensor(
                    out=scores, in0=scores, in1=mx.broadcast_to([P, WPC, 4]),
                    op=mybir.AluOpType.subtract)
                nc.scalar.activation(out=scores, in_=scores,
                                     func=mybir.ActivationFunctionType.Exp)
                nc.vector.reduce_sum(out=ssum, in_=scores,
                                     axis=mybir.AxisListType.X)
                nc.vector.reciprocal(out=rec, in_=ssum)
                nc.vector.tensor_tensor(
                    out=scores, in0=scores, in1=rec.broadcast_to([P, WPC, 4]),
                    op=mybir.AluOpType.mult)
                nc.vector.tensor_tensor(
                    out=acc0, in0=stack[:, :, 0, :],
                    in1=scores[:, :, 0:1].broadcast_to([P, WPC, D]),
                    op=mybir.AluOpType.mult)
                for k in range(1, 4):
                    nc.vector.tensor_tensor(
                        out=tmp, in0=stack[:, :, k, :],
                        in1=scores[:, :, k:k + 1].broadcast_to([P, WPC, D]),
                        op=mybir.AluOpType.mult)
                    nc.vector.tensor_tensor(out=acc0, in0=acc0, in1=tmp,
                                            op=mybir.AluOpType.add)
                nc.vector.tensor_tensor(out=acc0, in0=acc0, in1=res_ap,
                                        op=mybir.AluOpType.add)
                b = bias_sb[:, bias_col * D:(bias_col + 1) * D]
                nc.vector.tensor_tensor(
                    out=out_tile[:], in0=acc0,
                    in1=b.unsqueeze(1).broadcast_to([P, WPC, D]),
                    op=mybir.AluOpType.add)

            # ---------------- layer 1 ----------------
            dense_layer1()
            do_allgather(cc1[0], tabs1[0], stack1, 0, True)
            hop(1, tabs1[0], stack1, 1)
            do_allgather(cc1[1], tabs1[1], stack1, 1, True)
            hop(1, tabs1[1], stack1, 2)
            do_allgather(cc1[2], tabs1[2], stack1, 2, True)
            sm1 = patt.tile([P, WPC, 8], F32, tag="attsm", name="sm1")
            attention_early(stack1, stack2, 0, sm1)
            hop(1, tabs1[2], stack1, 3)

            # swap dstw schedule for layer 2 (overlaps with attention)
            nc.sync.dma_start(out=dstw_sb[:, 0:CH2], in_=dstwp2[:])

            attention(stack1, stack2, res1_sb[:], 0, g_sb, sm1)
            gm = stack2[:, :, 2, :]
            nc.vector.tensor_scalar_min(out=gm, in0=g_sb[:], scalar1=0.0)
            nc.scalar.activation(out=gm, in_=gm,
                                 func=mybir.ActivationFunctionType.Exp)
            gp = stack2[:, :, 3, :]
            nc.vector.tensor_scalar_max(out=gp, in0=g_sb[:], scalar1=0.0)
            nc.vector.scalar_tensor_tensor(
                out=g_sb[:], in0=gm, scalar=-1.0, in1=gp,
                op0=mybir.AluOpType.add, op1=mybir.AluOpType.add)

            # ---------------- layer 2 ----------------
            for t in range(WPC):
                pst = ppsd.tile([D, P], F32, tag="pd", name="pst")
                nc.tensor.transpose(out=pst[:], in_=g_sb[:, t, :],
                                    identity=ident_sb[:])
                nc.vector.tensor_copy(out=gT_sb[:, t, :], in_=pst[:])
            for t in range(WPC):
                ps = ppsd.tile([P, D], F32, tag="pd", name="ps2")
                nc.tensor.matmul(out=ps[:], lhsT=gT_sb[:, t, :],
                                 rhs=w2t_sb[:], start=True, stop=True)
                nc.vector.tensor_copy(out=stack2[:, t, 0, :], in_=ps[:])
            do_allgather(cc2[0], tabs2[0], stack2, 0, False)
            hop(2, tabs2[0], stack2, 1)
            do_allgather(cc2[1], tabs2[1], stack2, 1, False)
            hop(2, tabs2[1], stack2, 2)
            do_allgather(cc2[2], tabs2[2], stack2, 2, False)
            sm2 = patt.tile([P, WPC, 8], F32, tag="attsm", name="sm2")
            attention_early(stack2, stack1, 1, sm2)
            hop(2, tabs2[2], stack2, 3)

            out2_sb = pout.tile([P, WPC, D], F32, tag="outl", name="out2_sb")
            attention(stack2, stack1, stack2[:, :, 0, :], 1, out2_sb, sm2)
            nc.sync.dma_start(
                out=outp[:].rearrange("(p w) d -> p w d", p=P),
                in_=out2_sb[:],
            )
    nc.compile()
    return nc


_CACHE = {}
_last_in_maps = None


def kernel(**inputs):
    x = np.asarray(inputs["x"], dtype=np.float32)
    edge_index = np.asarray(inputs["edge_index"])
    W1 = np.asarray(inputs["W1"], dtype=np.float32)
    att1 = np.asarray(inputs["att1"], dtype=np.float32)
    bias1 = np.asarray(inputs["bias1"], dtype=np.float32)
    resW1 = np.asarray(inputs["resW1"], dtype=np.float32)
    W2 = np.asarray(inputs["W2"], dtype=np.float32)
    att2 = np.asarray(inputs["att2"], dtype=np.float32)
    bias2 = np.asarray(inputs["bias2"], dtype=np.float32)

    s1, s2 = _preprocess(edge_index)
    CH1, CH2 = s1[0], s2[0]

    key = ("k", CH1, CH2, tuple(s1[1][:4]), tuple(s2[1][:4]))
    if key not in _CACHE:
        _CACHE[key] = _build(s1, s2)
    nc = _CACHE[key]

    xpad = np.zeros((NPAD, DIN), dtype=np.float32)
    xpad[:N] = x
    iota_np = np.tile(np.arange(P, dtype=np.float32)[None, :], (P, 1)).astype(BF)
    ident_np = np.eye(P, dtype=np.float32)
    att_np = np.concatenate([
        np.tile(att1[0, 0, :D][None, :], (P, 1)),
        np.tile(att1[0, 0, D:][None, :], (P, 1)),
        np.tile(att2[0, 0, :D][None, :], (P, 1)),
        np.tile(att2[0, 0, D:][None, :], (P, 1)),
    ], axis=1).astype(np.float32)
    bias_np = np.concatenate([
        np.tile(bias1[None, :], (P, 1)),
        np.tile(bias2[None, :], (P, 1)),
    ], axis=1).astype(np.float32)
    w1t = np.ascontiguousarray(W1.T)
    rw1t = np.ascontiguousarray(resW1.T)
    w2t = np.ascontiguousarray(W2.T)

    jj = np.arange(NB)
    real_in_block = (jj % WPC) * P + (jj // WPC)

    in_maps = []
    for c in range(NCORES):
        xT_c = np.ascontiguousarray(
            xpad[c * NB:(c + 1) * NB].T.reshape(P, WPC, P).transpose(1, 0, 2))
        in_maps.append({
            "xT": xT_c,
            "idxp1": s1[3][c], "dstwp1": s1[4][c],
            "idxp2": s2[3][c], "dstwp2": s2[4][c],
            "iotap": iota_np, "identp": ident_np,
            "w1tp": w1t, "rw1tp": rw1t, "w2tp": w2t,
            "attp": att_np, "biasp": bias_np,
        })

    global _last_in_maps
    _last_in_maps = in_maps
    res = run_bass_kernel_spmd(nc, in_maps, core_ids=list(range(NCORES)))
    out = np.empty((NPAD, D), dtype=np.float32)
    for c in range(NCORES):
        out[c * NB + real_in_block] = res.results[c]["out"]
    return out[:N].astype(np.float32)


# revision 38
# speedup vs baseline: 1.1050x; 1.0544x over previous
"""AGDN (2-layer, K=3 hop) GNN message passing on 8 TRN2 NeuronCores.

Strategy (node-sharded, replicated feature table):
  - Nodes padded to 50176 = 8 * 6272; core c owns dst rows [c*6272, (c+1)*6272).
  - Per hop, each core processes the ~100K edges whose dst is in its block:
    SWDGE dma_gather pieces pull f[src] from the core's HBM copy of the
    full feature table (layer 1: f32 rows, 256B elems, int16 indices made
    valid by splitting each piece into two gathers over the two table
    halves; layer 2: bf16 node-pair 256B elems with parity-sliced rhs).
    PE segment-matmuls (one-hot S built on-chip by DVE is_equal vs iota)
    accumulate messages into per-128-dst-window PSUM tiles; windows are
    chunk-consecutive so each window drains once via ACT into the local
    f32 hop stack; an 8-core AllGather rebuilds the replicated table.
  - Dense layers, hop-attention softmax and ELU run locally per shard.
  - All edge sorting / chunk scheduling happens on host in numpy; the
    schedule is shared by all 8 cores (one SPMD graph), padded per
    (window, half/parity) group to the max count over cores.
"""

import numpy as np
import ml_dtypes

import concourse.bacc as bacc
import concourse.mybir as mybir
import concourse.tile as tile
from concourse.bass_utils import run_bass_kernel_spmd

N = 50000
E = 800000
DIN = 128
D = 64
NCORES = 8
P = 128
WPC = 49                  # 128-dst windows per core
NB = WPC * P              # 6272 nodes per core
NPAD = NCORES * NB        # 50176
NHALF = NPAD // 2
NG = WPC * 2
PIECE = 16                # gather chunks per piece
SBATCH = 8                # chunks per DVE S-build op
SLOPE = 0.2

F32 = mybir.dt.float32
BF16 = mybir.dt.bfloat16
I16 = mybir.dt.int16
BF = ml_dtypes.bfloat16


def _schedule(src_key, wloc, dstw, core, m_idx, split_keys=True):
    """Build one layer's schedule.

    src_key: 0/1 per edge (table half for L1, pair parity for L2)
    m_idx:   int16-safe gather index per edge
    Returns (CH, chunk_meta, pieces, idx_all, dstw_all) where chunk_meta
    entries are (window, key, first, last, slotcol).
    """
    g_in_core = wloc * 2 + src_key
    gkey = core * NG + g_in_core
    order = np.argsort(gkey, kind="stable")
    gk_sorted = gkey[order]
    run_start = np.searchsorted(gk_sorted, gk_sorted)
    pos_in_group = np.arange(E) - run_start
    inv = np.empty(E, dtype=np.int64)
    inv[order] = np.arange(E)
    pos_of_edge = pos_in_group[inv]

    cnt = np.bincount(gkey, minlength=NCORES * NG).reshape(NCORES, NG)
    gmax = cnt.max(axis=0)
    nchunks = np.ceil(gmax / P).astype(np.int64)   # per (w,key): g = 2w+key
    for w in range(WPC):
        if nchunks[2 * w] + nchunks[2 * w + 1] == 0:
            nchunks[2 * w] = 1
    CH = int(nchunks.sum())

    # program order: window-major, key-minor
    prog = []              # (w, key) per chunk
    first_chunk_of_group = {}
    for w in range(WPC):
        for key in (0, 1):
            if nchunks[2 * w + key]:
                first_chunk_of_group[(w, key)] = len(prog)
            for _ in range(int(nchunks[2 * w + key])):
                prog.append((w, key))

    # pieces + slot column assignment (key-0 chunks first within piece)
    cuts = []
    c0 = 0
    while c0 < CH:
        cuts.append((c0, min(c0 + PIECE, CH)))
        c0 += PIECE
    if len(cuts) > 1:
        lo, hi = cuts[-1]
        cuts = cuts[:-1]
        n = hi - lo
        if n > 16:
            cuts += [(lo, lo + n - 16), (lo + n - 16, lo + n - 8),
                     (lo + n - 8, hi)]
        elif n > 8:
            cuts += [(lo, lo + n - 8), (lo + n - 8, hi)]
        else:
            cuts.append((lo, hi))
    pieces = []            # (col0, [(key, ncols)...])
    col_of_chunk = [0] * CH
    for (c0, c1) in cuts:
        if not split_keys:
            for c in range(c0, c1):
                col_of_chunk[c] = c
            pieces.append((c0, [(0, c1 - c0)]))
            continue
        ids0 = [c for c in range(c0, c1) if prog[c][1] == 0]
        ids1 = [c for c in range(c0, c1) if prog[c][1] == 1]
        col = c0
        subs = []
        for key, ids in ((0, ids0), (1, ids1)):
            if ids:
                subs.append((key, len(ids)))
                for c in ids:
                    col_of_chunk[c] = col
                    col += 1
        pieces.append((c0, subs))

    # chunk meta with per-window first/last accumulation flags
    wtot = [0] * WPC
    for w, key in prog:
        wtot[w] += 1
    wseen = [0] * WPC
    chunk_meta = []
    for c, (w, key) in enumerate(prog):
        chunk_meta.append((w, key, wseen[w] == 0, wseen[w] == wtot[w] - 1,
                           col_of_chunk[c]))
        wseen[w] += 1

    # per-core slot arrays
    gfirst = np.zeros(NG, dtype=np.int64)
    for (w, key), fc in first_chunk_of_group.items():
        gfirst[2 * w + key] = fc
    col_arr = np.asarray(col_of_chunk, dtype=np.int64)
    idx_all, dstw_all = [], []
    nslots = CH * P
    for c in range(NCORES):
        mask = core == c
        g = g_in_core[mask]
        pos = pos_of_edge[mask]
        ch = gfirst[g] + (pos >> 7)
        slot = col_arr[ch] * P + (pos & 127)
        iarr = np.zeros(nslots, dtype=np.int16)
        darr = np.full(nslots, -1.0, dtype=np.float32)
        iarr[slot] = m_idx[mask].astype(np.int16)
        darr[slot] = dstw[mask].astype(np.float32)
        wrapped = np.tile(iarr.reshape(CH * 8, 16).T, (8, 1))
        idx_all.append(np.ascontiguousarray(wrapped))
        dstw_all.append(np.ascontiguousarray(
            darr.reshape(CH, P).T.astype(BF)))
    return CH, chunk_meta, pieces, idx_all, dstw_all


def _preprocess(edge_index):
    src = np.ascontiguousarray(edge_index[0]).astype(np.int64)
    dst = np.ascontiguousarray(edge_index[1]).astype(np.int64)
    core = dst // NB
    wloc = (dst % NB) >> 7
    dstw = dst & 127
    sc = src // NB
    sr = src % NB
    m_src = sc * NB + (sr & 127) * WPC + (sr >> 7)   # device node id

    half = (m_src >= NHALF).astype(np.int64)
    s1 = _schedule(half, wloc, dstw, core, m_src - half * NHALF)
    par = m_src & 1
    s2 = _schedule(par, wloc, dstw, core, m_src >> 1)
    return s1, s2


def _build(s1_struct, s2_struct):
    CH1, meta1, pieces1, _, _ = s1_struct
    CH2, meta2, pieces2, _, _ = s2_struct
    CHmax = max(CH1, CH2)

    nc = bacc.Bacc("TRN2", target_bir_lowering=False, debug=False,
                   num_devices=NCORES, num_swdge_queues=4)

    with tile.TileContext(nc) as tc:
        xT = nc.dram_tensor("xT", [WPC, P, P], F32, kind="ExternalInput")
        idxp1 = nc.dram_tensor("idxp1", [P, CH1 * 8], I16, kind="ExternalInput")
        dstwp1 = nc.dram_tensor("dstwp1", [P, CH1], BF16, kind="ExternalInput")
        idxp2 = nc.dram_tensor("idxp2", [P, CH2 * 8], I16, kind="ExternalInput")
        dstwp2 = nc.dram_tensor("dstwp2", [P, CH2], BF16, kind="ExternalInput")
        iotap = nc.dram_tensor("iotap", [P, P], BF16, kind="ExternalInput")
        identp = nc.dram_tensor("identp", [P, P], F32, kind="ExternalInput")
        w1tp = nc.dram_tensor("w1tp", [P, D], F32, kind="ExternalInput")
        rw1tp = nc.dram_tensor("rw1tp", [P, D], F32, kind="ExternalInput")
        w2tp = nc.dram_tensor("w2tp", [D, D], F32, kind="ExternalInput")
        attp = nc.dram_tensor("attp", [P, 4 * D], F32, kind="ExternalInput")
        biasp = nc.dram_tensor("biasp", [P, 2 * D], F32, kind="ExternalInput")
        outp = nc.dram_tensor("out", [NB, D], F32, kind="ExternalOutput")

        cc1 = [nc.dram_tensor(f"ccA{i}", [NB, D], F32) for i in range(3)]
        tabs1 = [nc.dram_tensor(f"tabA{i}", [NPAD, D], F32,
                                addr_space="Shared") for i in range(3)]
        cc2 = [nc.dram_tensor(f"ccB{i}", [NB, D], BF16) for i in range(3)]
        tabs2 = [nc.dram_tensor(f"tabB{i}", [NPAD, D], BF16,
                                addr_space="Shared") for i in range(3)]

        with tc.tile_pool(name="const", bufs=1) as pconst, \
             tc.tile_pool(name="idxpool", bufs=1) as pidx, \
             tc.tile_pool(name="big", bufs=1) as pbig, \
             tc.tile_pool(name="xt", bufs=3) as pxt, \
             tc.tile_pool(name="idxt", bufs=8) as pidx2, \
             tc.tile_pool(name="outl", bufs=1) as pout, \
             tc.tile_pool(name="g2", bufs=8) as pg2, \
             tc.tile_pool(name="sseg", bufs=3) as pseg, \
             tc.tile_pool(name="att", bufs=1) as patt, \
             tc.tile_pool(name="psum", bufs=4, space="PSUM") as pps, \
             tc.tile_pool(name="psumd", bufs=2, space="PSUM") as ppsd:

            iota_sb = pconst.tile([P, P], BF16)
            nc.sync.dma_start(out=iota_sb[:], in_=iotap[:])
            ident_sb = pconst.tile([P, P], F32)
            nc.sync.dma_start(out=ident_sb[:], in_=identp[:])
            w1t_sb = pconst.tile([P, D], F32)
            nc.sync.dma_start(out=w1t_sb[:], in_=w1tp[:])
            rw1t_sb = pconst.tile([P, D], F32)
            nc.sync.dma_start(out=rw1t_sb[:], in_=rw1tp[:])
            w2t_sb = pconst.tile([D, D], F32)
            nc.sync.dma_start(out=w2t_sb[:], in_=w2tp[:])
            att_sb = pconst.tile([P, 4 * D], F32)
            nc.sync.dma_start(out=att_sb[:], in_=attp[:])
            bias_sb = pconst.tile([P, 2 * D], F32)
            nc.sync.dma_start(out=bias_sb[:], in_=biasp[:])

            dstw_sb = pidx.tile([P, CHmax], BF16)
            nc.sync.dma_start(out=dstw_sb[:, 0:CH1], in_=dstwp1[:])

            stack1 = pbig.tile([P, WPC, 4, D], F32)
            stack2 = pbig.tile([P, WPC, 4, D], F32)
            res1_sb = pbig.tile([P, WPC, D], F32)
            gT_sb = pbig.tile([D, WPC, P], F32)
            g_sb = pout.tile([P, WPC, D], F32, tag="outl", name="g_sb")

            def dense_layer1():
                for t in range(WPC):
                    xtile = pxt.tile([P, P], F32, tag="xt", name="xtile")
                    nc.sync.dma_start(out=xtile[:], in_=xT[t, :, :])
                    ps = ppsd.tile([P, D], F32, tag="pd")
                    nc.tensor.matmul(out=ps[:], lhsT=xtile[:], rhs=w1t_sb[:],
                                     start=True, stop=True)
                    nc.vector.tensor_copy(out=stack1[:, t, 0, :], in_=ps[:])
                    ps2 = ppsd.tile([P, D], F32, tag="pd")
                    nc.tensor.matmul(out=ps2[:], lhsT=xtile[:], rhs=rw1t_sb[:],
                                     start=True, stop=True)
                    nc.scalar.copy(out=res1_sb[:, t, :], in_=ps2[:])

            def do_allgather(cc, tab, stack, k, f32):
                eng = nc.sync if f32 else nc.gpsimd   # bf16 needs cast DMA
                eng.dma_start(
                    out=cc[:].rearrange("(p w) d -> p w d", p=P),
                    in_=stack[:, :, k, :],
                )
                nc.gpsimd.collective_compute(
                    "AllGather", mybir.AluOpType.bypass,
                    replica_groups=[list(range(NCORES))],
                    ins=[cc[:].opt()],
                    outs=[tab[:].opt()],
                )

            qctr = [0]

            def hop(layer, tab, stack, k):
                """stack[:, :, k, :] = A @ table."""
                if layer == 1:
                    meta, pieces, sdt, gcols = meta1, pieces1, F32, D
                    idxp = idxp1
                    bases = [tab[0:NHALF, :], tab[NHALF:NPAD, :]]
                else:
                    meta, pieces, sdt, gcols = meta2, pieces2, BF16, 2 * D
                    idxp = idxp2
                    bases = [tab[:].rearrange("(a b) d -> a (b d)", b=2)] * 2
                pw = None
                for (pc0, subs) in pieces:
                    npc = sum(n for _, n in subs)
                    g2 = pg2.tile([P, PIECE, gcols], sdt if gcols == 2 * D else F32, tag="g2", name="g2")
                    idxt = pidx2.tile([P, PIECE * 8], I16, tag="idxt",
                                      name="idxt")
                    nc.sync.dma_start(out=idxt[:, 0:npc * 8],
                                      in_=idxp[:, pc0 * 8:(pc0 + npc) * 8])
                    scol = 0
                    for key, ncol in subs:
                        nc.gpsimd.dma_gather(
                            g2[:, scol:scol + ncol, :], bases[key],
                            idxt[:, scol * 8:(scol + ncol) * 8],
                            ncol * P, ncol * P, gcols,
                            single_packet=False, queue_num=qctr[0] % 4,
                        )
                        qctr[0] += 1
                        scol += ncol
                    sb_tiles = {}
                    for b0 in range(0, npc, SBATCH):
                        b1 = min(b0 + SBATCH, npc)
                        st = pseg.tile([P, SBATCH * P], sdt, tag="sseg",
                                       name="st")
                        nc.vector.tensor_tensor(
                            out=st[:, 0:(b1 - b0) * P].rearrange(
                                "p (c q) -> p c q", q=P),
                            in0=dstw_sb[:, pc0 + b0:pc0 + b1].to_broadcast(
                                [P, b1 - b0, P]),
                            in1=iota_sb[:].unsqueeze(1).broadcast_to(
                                [P, b1 - b0, P]),
                            op=mybir.AluOpType.is_equal,
                        )
                        sb_tiles[b0] = st
                    for c in range(pc0, pc0 + npc):
                        w, key, first, last, col = meta[c]
                        if first:
                            pw = pps.tile([P, D], F32, tag="pwin", name="pw")
                        lcol = col - pc0
                        st = sb_tiles[(lcol // SBATCH) * SBATCH]
                        j = lcol % SBATCH
                        if layer == 1:
                            rhs = g2[:, lcol, :]
                        else:
                            rhs = g2[:, lcol, key * D:(key + 1) * D]
                        nc.tensor.matmul(
                            out=pw[:], lhsT=st[:, j * P:(j + 1) * P],
                            rhs=rhs, start=first, stop=last,
                        )
                        if last:
                            nc.scalar.copy(out=stack[:, w, k, :], in_=pw[:])

            def attention_early(stack, scratch, bias_col, sm):
                """Score terms that only need hops 0..2 (run before hop 3)."""
                a_hop = att_sb[:, (2 * bias_col + 1) * D:(2 * bias_col + 2) * D]
                a_h0 = att_sb[:, (2 * bias_col) * D:(2 * bias_col + 1) * D]
                tmp = scratch[:, :, 0, :]
                scf = sm[:, :, 0:4]
                sc0 = sm[:, :, 4:5]
                for k in range(3):
                    nc.vector.tensor_tensor(
                        out=tmp, in0=stack[:, :, k, :],
                        in1=a_hop.unsqueeze(1).broadcast_to([P, WPC, D]),
                        op=mybir.AluOpType.mult)
                    nc.vector.reduce_sum(out=scf[:, :, k:k + 1], in_=tmp,
                                         axis=mybir.AxisListType.X)
                nc.vector.tensor_tensor(
                    out=tmp, in0=stack[:, :, 0, :],
                    in1=a_h0.unsqueeze(1).broadcast_to([P, WPC, D]),
                    op=mybir.AluOpType.mult)
                nc.vector.reduce_sum(out=sc0, in_=tmp,
                                     axis=mybir.AxisListType.X)

            def attention(stack, scratch, res_ap, bias_col, out_tile, sm):
                a_hop = att_sb[:, (2 * bias_col + 1) * D:(2 * bias_col + 2) * D]
                tmp = scratch[:, :, 0, :]
                acc0 = scratch[:, :, 1, :]
                scf = sm[:, :, 0:4]
                scores = sm[:, :, 0:4]
                sc0 = sm[:, :, 4:5]
                mx = sm[:, :, 5:6]
                ssum = sm[:, :, 6:7]
                rec = sm[:, :, 7:8]
                nc.vector.tensor_tensor(
                    out=tmp, in0=stack[:, :, 3, :],
                    in1=a_hop.unsqueeze(1).broadcast_to([P, WPC, D]),
                    op=mybir.AluOpType.mult)
                nc.vector.reduce_sum(out=scf[:, :, 3:4], in_=tmp,
                                     axis=mybir.AxisListType.X)
                nc.vector.tensor_tensor(
                    out=scores, in0=scf, in1=sc0.broadcast_to([P, WPC, 4]),
                    op=mybir.AluOpType.add)
                nc.vector.scalar_tensor_tensor(
                    out=scores, in0=scores, scalar=SLOPE, in1=scores,
                    op0=mybir.AluOpType.mult, op1=mybir.AluOpType.max)
                nc.vector.reduce_max(out=mx, in_=scores,
                                     axis=mybir.AxisListType.X)
                nc.vector.tensor_tensor(
                    out=scores, in0=scores, in1=mx.broadcast_to([P, WPC, 4]),
                    op=mybir.AluOpType.subtract)
                nc.scalar.activation(out=scores, in_=scores,
                                     func=mybir.ActivationFunctionType.Exp)
                nc.vector.reduce_sum(out=ssum, in_=scores,
                                     axis=mybir.AxisListType.X)
                nc.vector.reciprocal(out=rec, in_=ssum)
                nc.vector.tensor_tensor(
                    out=scores, in0=scores, in1=rec.broadcast_to([P, WPC, 4]),
                    op=mybir.AluOpType.mult)
                nc.vector.tensor_tensor(
                    out=acc0, in0=stack[:, :, 0, :],
                    in1=scores[:, :, 0:1].broadcast_to([P, WPC, D]),
                    op=mybir.AluOpType.mult)
                for k in range(1, 4):
                    nc.vector.tensor_tensor(
                        out=tmp, in0=stack[:, :, k, :],
                        in1=scores[:, :, k:k + 1].broadcast_to([P, WPC, D]),
                        op=mybir.AluOpType.mult)
                    nc.vector.tensor_tensor(out=acc0, in0=acc0, in1=tmp,
                                            op=mybir.AluOpType.add)
                nc.vector.tensor_tensor(out=acc0, in0=acc0, in1=res_ap,
                                        op=mybir.AluOpType.add)
                b = bias_sb[:, bias_col * D:(bias_col + 1) * D]
                nc.vector.tensor_tensor(
                    out=out_tile[:], in0=acc0,
                    in1=b.unsqueeze(1).broadcast_to([P, WPC, D]),
                    op=mybir.AluOpType.add)

            # ---------------- layer 1 ----------------
            dense_layer1()
            do_allgather(cc1[0], tabs1[0], stack1, 0, True)
            hop(1, tabs1[0], stack1, 1)
            do_allgather(cc1[1], tabs1[1], stack1, 1, True)
            hop(1, tabs1[1], stack1, 2)
            do_allgather(cc1[2], tabs1[2], stack1, 2, True)
            sm1 = patt.tile([P, WPC, 8], F32, tag="attsm", name="sm1")
            attention_early(stack1, stack2, 0, sm1)
            hop(1, tabs1[2], stack1, 3)

            # swap dstw schedule for layer 2 (overlaps with attention)
            nc.sync.dma_start(out=dstw_sb[:, 0:CH2], in_=dstwp2[:])

            attention(stack1, stack2, res1_sb[:], 0, g_sb, sm1)
            gm = stack2[:, :, 2, :]
            nc.vector.tensor_scalar_min(out=gm, in0=g_sb[:], scalar1=0.0)
            nc.scalar.activation(out=gm, in_=gm,
                                 func=mybir.ActivationFunctionType.Exp)
            gp = stack2[:, :, 3, :]
            nc.vector.tensor_scalar_max(out=gp, in0=g_sb[:], scalar1=0.0)
            nc.vector.scalar_tensor_tensor(
                out=g_sb[:], in0=gm, scalar=-1.0, in1=gp,
                op0=mybir.AluOpType.add, op1=mybir.AluOpType.add)

            # ---------------- layer 2 ----------------
            for t in range(WPC):
                pst = ppsd.tile([D, P], F32, tag="pd", name="pst")
                nc.tensor.transpose(out=pst[:], in_=g_sb[:, t, :],
                                    identity=ident_sb[:])
                nc.vector.tensor_copy(out=gT_sb[:, t, :], in_=pst[:])
            for t in range(WPC):
                ps = ppsd.tile([P, D], F32, tag="pd", name="ps2")
                nc.tensor.matmul(out=ps[:], lhsT=gT_sb[:, t, :],
                                 rhs=w2t_sb[:], start=True, stop=True)
                nc.vector.tensor_copy(out=stack2[:, t, 0, :], in_=ps[:])
            do_allgather(cc2[0], tabs2[0], stack2, 0, False)
            hop(2, tabs2[0], stack2, 1)
            do_allgather(cc2[1], tabs2[1], stack2, 1, False)
            hop(2, tabs2[1], stack2, 2)
            do_allgather(cc2[2], tabs2[2], stack2, 2, False)
            sm2 = patt.tile([P, WPC, 8], F32, tag="attsm", name="sm2")
            attention_early(stack2, stack1, 1, sm2)
            hop(2, tabs2[2], stack2, 3)

            out2_sb = pout.tile([P, WPC, D], F32, tag="outl", name="out2_sb")
            attention(stack2, stack1, stack2[:, :, 0, :], 1, out2_sb, sm2)
            nc.sync.dma_start(
                out=outp[:].rearrange("(p w) d -> p w d", p=P),
                in_=out2_sb[:],
            )
    nc.compile()
    return nc


_CACHE = {}
_last_in_maps = None


def kernel(**inputs):
    x = np.asarray(inputs["x"], dtype=np.float32)
    edge_index = np.asarray(inputs["edge_index"])
    W1 = np.asarray(inputs["W1"], dtype=np.float32)
    att1 = np.asarray(inputs["att1"], dtype=np.float32)
    bias1 = np.asarray(inputs["bias1"], dtype=np.float32)
    resW1 = np.asarray(inputs["resW1"], dtype=np.float32)
    W2 = np.asarray(inputs["W2"], dtype=np.float32)
    att2 = np.asarray(inputs["att2"], dtype=np.float32)
    bias2 = np.asarray(inputs["bias2"], dtype=np.float32)

    s1, s2 = _preprocess(edge_index)
    CH1, CH2 = s1[0], s2[0]

    key = ("k", CH1, CH2, tuple(s1[1][:4]), tuple(s2[1][:4]))
    if key not in _CACHE:
        _CACHE[key] = _build(s1, s2)
    nc = _CACHE[key]

    xpad = np.zeros((NPAD, DIN), dtype=np.float32)
    xpad[:N] = x
    iota_np = np.tile(np.arange(P, dtype=np.float32)[None, :], (P, 1)).astype(BF)
    ident_np = np.eye(P, dtype=np.float32)
    att_np = np.concatenate([
        np.tile(att1[0, 0, :D][None, :], (P, 1)),
        np.tile(att1[0, 0, D:][None, :], (P, 1)),
        np.tile(att2[0, 0, :D][None, :], (P, 1)),
        np.tile(att2[0, 0, D:][None, :], (P, 1)),
    ], axis=1).astype(np.float32)
    bias_np = np.concatenate([
        np.tile(bias1[None, :], (P, 1)),
        np.tile(bias2[None, :], (P, 1)),
    ], axis=1).astype(np.float32)
    w1t = np.ascontiguousarray(W1.T)
    rw1t = np.ascontiguousarray(resW1.T)
    w2t = np.ascontiguousarray(W2.T)

    jj = np.arange(NB)
    real_in_block = (jj % WPC) * P + (jj // WPC)

    in_maps = []
    for c in range(NCORES):
        xT_c = np.ascontiguousarray(
            xpad[c * NB:(c + 1) * NB].T.reshape(P, WPC, P).transpose(1, 0, 2))
        in_maps.append({
            "xT": xT_c,
            "idxp1": s1[3][c], "dstwp1": s1[4][c],
            "idxp2": s2[3][c], "dstwp2": s2[4][c],
            "iotap": iota_np, "identp": ident_np,
            "w1tp": w1t, "rw1tp": rw1t, "w2tp": w2t,
            "attp": att_np, "biasp": bias_np,
        })

    global _last_in_maps
    _last_in_maps = in_maps
    res = run_bass_kernel_spmd(nc, in_maps, core_ids=list(range(NCORES)))
    out = np.empty((NPAD, D), dtype=np.float32)
    for c in range(NCORES):
        out[c * NB + real_in_block] = res.results[c]["out"]
    return out[:N].astype(np.float32)
